# revision 108
# baseline (speedup 1.0000x reference)
"""Causal single-head attention (B=4, T=4096, C=512, H=64) on 8 trn2 NeuronCores.

Sharding: core (2b+par) handles batch b and the query 128-row blocks with
parity `par` (even/odd interleave) -> perfectly balanced causal work, all 8
cores run an IDENTICAL program (SPMD); cross-core differences live in data.

k-block permutation trick: softmax over k is order-invariant, so the host
hands each core xT with its OWN parity blocks packed first (cols 0:2048) and
the other parity's blocks second (cols 2048:4096).  Then:
  - Q^T projection reads compile-time slices (cols 0:2048) -- no extra load
  - k-tile t < 16 ("own") has its causal diagonal at q-block t  -> tri mask
  - k-tile 16+s ("other") starts at q-block s; its first block is fully
    masked (par=0) or fully visible (par=1) -> data mask zeros/ones
Device dataflow per core (bf16 matmuls, f32 accumulation):
  Q^T = Wq^T x_own^T        [64, 2048]
  [K^T;V^T] = [Wk|Wv]^T x^T [128, 4096]
  V natural per k-tile via identity matmul; col 64 = 1.0 (row-sum trick)
  scores: k-tiles bin-packed so each exp activation covers ~1024 columns
  (causal taper pairs to exactly 1024); S^T matmul -> exp -> mask -> P^T
  per (k-tile, q-block): av[128q, 65] += P_block^T-as-weights @ V_tile,
  one PSUM accumulation group per 4-block sub-chunk bank [128, 4, 65]
  normalize rows by 1/av[:,:,64]; output natural [q, 64] bf16, host
  unshuffles.  Projections/transposes stream into the PE queue as
  deadline-fillers so DMA, PE and ACT pipelines overlap end to end.
"""

import math

import numpy as np
import ml_dtypes

T = 4096
C = 512
H = 64
B = 4
NCORES = 8
TQ = T // 2          # own query rows per core
NJ = TQ // 128       # 16 own q blocks
NK = T // 128        # 32 k tiles
NT = NK // 2         # 16 own (or other) k tiles
CHUNK = 1024         # q columns per processing chunk
NCH = TQ // CHUNK    # 2 chunks
NBL = CHUNK // 128   # 8 q blocks per chunk

BF16 = ml_dtypes.bfloat16

WARMUP_MM = 29       # PE p-state warmup matmuls (tuned against TimelineSim)

_PROGRAM_CACHE = {}


def build_program():
    import concourse.bass as bass
    import concourse.mybir as mybir
    from concourse import bacc
    from concourse.tile import TileContext
    from concourse.masks import make_identity

    f32 = mybir.dt.float32
    bf16 = mybir.dt.bfloat16

    nc = bacc.Bacc(
        "TRN2", target_bir_lowering=False, debug=False, num_devices=NCORES
    )

    xT_d = nc.dram_tensor("xT", [128, T * 4], bf16, kind="ExternalInput").ap()
    w_d = nc.dram_tensor("w", [128, 4 * 192], bf16, kind="ExternalInput").ap()
    msk_d = nc.dram_tensor("msk", [128, 256], bf16, kind="ExternalInput").ap()
    out_d = nc.dram_tensor("out", [128, NJ * H], bf16, kind="ExternalOutput").ap()

    EXP = mybir.ActivationFunctionType.Exp
    inv_sqrt_c = 1.0 / math.sqrt(C)

    with TileContext(nc) as tc:
        with (
            tc.tile_pool(name="const", bufs=1) as constp,
            tc.tile_pool(name="big", bufs=1) as bigp,
            tc.tile_pool(name="projp", bufs=2, space="PSUM") as projp,
            tc.tile_pool(name="stp", bufs=2, space="PSUM") as stp,
            tc.tile_pool(name="avp", bufs=2, space="PSUM") as avp,
            tc.tile_pool(name="ptp", bufs=8) as ptp,
            tc.tile_pool(name="rcpp", bufs=2) as rcpp,
        ):
            identb = constp.tile([128, 128], bf16)
            make_identity(nc, identb[:])

            # PE p-state warmup: the sim models a 3us ramp to full matmul
            # speed from the start of a continuous PE run, and idle gaps reset
            # it.  Keep PE busy with throwaway matmuls from ~0.5us until the
            # first xT chunk lands (~4.4us) so projections run at full speed.
            warm_src = constp.tile([64, 128], bf16)
            nc.gpsimd.memset(warm_src[:], 0.5)
            warm = stp.tile([128, CHUNK], f32, tag="st", name="warm")
            for _ in range(WARMUP_MM):
                nc.tensor.matmul(
                    warm[:, 0:128], warm_src[:], warm_src[:],
                    start=True, stop=True,
                )

            w_sb = constp.tile([128, 4, 192], bf16)
            msk_sb = constp.tile([128, 256], bf16)
            xT_sb = bigp.tile([128, T, 4], bf16)
            xT_r = xT_d.rearrange("p (t a) -> p t a", a=4)

            def xdma(lo, hi):
                sl = slice(lo, hi)
                nc.sync.dma_start(out=xT_sb[:, sl, :], in_=xT_r[:, sl, :])

            # DMA issue order: HWDGE triggers and transfers serialize, so the
            # startup-critical pieces (w for ldweights, then xT in 256-col
            # slivers) go first; the mask load hides behind later chunks.
            # xT streams in consumption order: own-c0, other-c0, own-c1,
            # other-c1.
            nc.sync.dma_start(
                out=w_sb[:], in_=w_d.rearrange("p (a h) -> p a h", a=4)
            )
            xdma(0, 256)
            xdma(256, 512)
            xdma(512, 768)
            xdma(768, 1024)
            nc.sync.dma_start(out=msk_sb[:], in_=msk_d)
            for tch in [4, 5, 2, 3, 6, 7]:
                xdma(tch * 512, (tch + 1) * 512)

            KVt = bigp.tile([128, T], bf16)   # rows 0:64 K^T, 64:128 V^T
            Qt = bigp.tile([64, TQ], bf16)    # own-parity Q^T
            Vn = bigp.tile([128, NK, H + 1], bf16)
            outsb = bigp.tile([128, NJ, H], bf16)
            # ones column for the row-sum trick
            nc.gpsimd.memset(Vn[:, :, H : H + 1], 1.0)

            def emit_kv(lo, hi):
                sl = slice(lo, hi)
                w = hi - lo
                ps = projp.tile([128, 512], f32, tag="scr")
                for a in range(4):
                    nc.tensor.matmul(
                        ps[:, 0:w], w_sb[:, a, 64:192], xT_sb[:, sl, a],
                        start=(a == 0), stop=(a == 3),
                    )
                nc.vector.tensor_copy(KVt[:, sl], ps[:, 0:w])

            def emit_kv_chunk(t):
                emit_kv(t * 512, (t + 1) * 512)

            def emit_q(lo, hi):
                sl = slice(lo, hi)
                w = hi - lo
                ps = projp.tile([128, 512], f32, tag="scr")
                for a in range(4):
                    nc.tensor.matmul(
                        ps[0:64, 0:w], w_sb[:, a, 0:64], xT_sb[:, sl, a],
                        start=(a == 0), stop=(a == 3),
                    )
                nc.vector.tensor_copy(Qt[:, sl], ps[0:64, 0:w])

            def emit_q_chunk(t):
                emit_q(t * 512, (t + 1) * 512)

            def emit_vtrans(t4, n=4):
                # transpose V^T for k-tiles t4..t4+n-1 into Vn
                ps = projp.tile([128, 4, 128], f32, tag="scr")
                for i in range(n):
                    tt = t4 + i
                    sl = slice(tt * 128, (tt + 1) * 128)
                    nc.tensor.matmul(
                        ps[:, i, 0:H],
                        KVt[64:128, sl], identb[64:128, 64:128],
                        start=True, stop=True,
                    )
                nc.vector.tensor_copy(
                    Vn[:, t4 : t4 + n, 0:H], ps[:, 0:n, 0:H]
                )

            emit_vtrans4 = emit_vtrans

            # ---- attention machinery ----
            av = {}       # chunk -> psum accumulator tile
            pend = []     # pending AV emissions (1-tile software pipeline)

            def flush_pend(keep=0):
                while len(pend) > keep:
                    pend.pop(0)()

            def emit_chunk(c, fillers):
                """Attention for chunk c.  Tiles are bin-packed so every exp
                activation covers (close to) CHUNK columns: the causal taper
                pairs width 1024-128*i with 1024-128*(8-i) to exactly 1024.
                AV start/stop flags are derived from actual emission order.
                fillers: (deadline_bin_idx, thunk) list of proj work emitted
                into the PE stream before that bin."""
                # one accumulator bank per 4-block sub-chunk: [128, 4, 65]
                # fits a 2KB PSUM bank with no matmul bank-straddle, so each
                # sub-chunk is a single accumulation group (start on first AV
                # touching it, stop on last; norms read only after the stop).
                avs = [
                    avp.tile([128, 4, H + 1], f32, tag="av", name="av")
                    for _ in range(NBL // 4)
                ]
                av[c] = avs

                # tile entries: (tile, b0, b1, mcol) covering rel q-blocks
                # [b0, b1).  For c0 the very first tile is split in two so the
                # first exp depends only on xT chunk 0 (Q cols 0:512).
                own = []
                for t in range(8 * c + 8):
                    b0 = max(t - 8 * c, 0)
                    mc = 0 if t >= 8 * c else None
                    if c == 0 and t == 0:
                        # split by DMA arrival: blocks 0-1 need only xT cols
                        # 0:256, 2-3 cols 256:512, 4-5 cols 512:768, 6-7 the
                        # rest -- the exp stream starts as data lands
                        own.append((0, 0, 2, 0))
                        own.append((0, 2, 4, None))
                        own.append((0, 4, 6, None))
                        own.append((0, 6, NBL, None))
                    else:
                        own.append((t, b0, NBL, mc))
                oth = []
                for s in range(8 * c + 8):
                    b0 = max(s - 8 * c, 0)
                    oth.append((NT + s, b0, NBL, 128 if s >= 8 * c else None))

                def pack(items, tail=False):
                    # widest-first two-pointer packing into <=CHUNK bins.
                    # tail=True: isolate the narrowest tile as the final
                    # single bin so the program-tail chain (exp -> AV -> norm
                    # -> out DMA) hangs off the smallest possible unit.
                    wid = lambda it: (it[2] - it[1]) * 128
                    items = sorted(items, key=lambda it: -wid(it))
                    tail_item = items.pop() if tail and len(items) > 1 else None
                    bins = []
                    lo, hi = 0, len(items) - 1
                    while lo <= hi:
                        binn = [items[lo]]
                        if lo < hi and wid(items[lo]) + wid(items[hi]) <= CHUNK:
                            binn.append(items[hi])
                            hi -= 1
                        bins.append(binn)
                        lo += 1
                    if tail_item is not None:
                        bins.append([tail_item])
                    return bins

                if c == 0:
                    # first four bins ordered by DMA arrival of their inputs
                    seq = [[e] for e in own[0:4]] + pack(own[4:]) + pack(oth)
                else:
                    seq = pack(own) + pack(oth, tail=False)

                # per-bank (sub-chunk) accumulation-group first/last AVs
                av_order = [
                    (bi, ti, jj)
                    for bi, binn in enumerate(seq)
                    for ti, (tile, b0, b1, mcol) in enumerate(binn)
                    for jj in range(b0, b1)
                ]
                bank_first, bank_last = {}, {}
                for key in av_order:
                    bk = key[2] // 4
                    if bk not in bank_first:
                        bank_first[bk] = key
                    bank_last[bk] = key
                stops_by_bin = {}
                for bk, (bi, ti, jj) in bank_last.items():
                    stops_by_bin.setdefault(bi, []).append(bk)

                fi = 0
                for bi, binn in enumerate(seq):
                    while fi < len(fillers) and fillers[fi][0] <= bi:
                        fillers[fi][1]()
                        fi += 1
                    st = stp.tile([128, CHUNK], f32, tag="st")
                    pt = ptp.tile([128, CHUNK], bf16, tag="pt")
                    off = 0
                    placed = []
                    for tile, b0, b1, mcol in binn:
                        w = (b1 - b0) * 128
                        ksl = slice(tile * 128, (tile + 1) * 128)
                        plo = off
                        while plo < off + w:  # split at PSUM bank boundaries
                            phi = min(off + w, (plo // 512 + 1) * 512)
                            q0 = c * CHUNK + b0 * 128 + (plo - off)
                            nc.tensor.matmul(
                                st[:, plo:phi], KVt[0:64, ksl],
                                Qt[:, q0 : q0 + (phi - plo)],
                                start=True, stop=True,
                            )
                            plo = phi
                        placed.append((tile, b0, b1, mcol, off))
                        off += w
                    nc.scalar.activation(
                        pt[:, 0:off], st[:, 0:off], EXP, scale=inv_sqrt_c
                    )
                    # program-tail masks go to the idle GpSimd engine so they
                    # don't queue behind norm work on DVE
                    meng = nc.gpsimd if (c == 1 and bi >= len(seq) - 2) else nc.vector
                    for tile, b0, b1, mcol, o in placed:
                        if mcol is not None:
                            meng.tensor_mul(
                                pt[:, o : o + 128], pt[:, o : o + 128],
                                msk_sb[:, mcol : mcol + 128],
                            )
                    flush_pend(keep=2)

                    def emit_av(bi=bi, placed=placed, pt=pt, avs=avs):
                        for ti, (tile, b0, b1, mcol, o) in enumerate(placed):
                            for jj in range(b0, b1):
                                po = o + (jj - b0) * 128
                                key = (bi, ti, jj)
                                bk = jj // 4
                                nc.tensor.matmul(
                                    avs[bk][:, jj % 4, :],
                                    pt[:, po : po + 128], Vn[:, tile, :],
                                    start=(key == bank_first[bk]),
                                    stop=(key == bank_last[bk]),
                                )
                        # when a sub-chunk's accumulation group closed, emit
                        # its norms (one batched reciprocal, 4 muls) and
                        # stream out that half of the chunk
                        for bk in stops_by_bin.get(bi, []):
                            j0 = c * NBL + bk * 4
                            rc = rcpp.tile([128, 4, 1], f32, tag="rc", name="rc")
                            nc.vector.reciprocal(rc[:, :, 0], avs[bk][:, :, H])
                            a2, r2 = bass.broadcast_tensor_aps(
                                avs[bk][:, :, 0:H], rc[:, :, 0:1]
                            )
                            nc.vector.tensor_mul(
                                outsb[:, j0 : j0 + 4, :], a2, r2
                            )
                            nc.sync.dma_start(
                                out=out_d[:, j0 * H : (j0 + 4) * H],
                                in_=outsb[:, j0 : j0 + 4, :],
                            )

                    pend.append(emit_av)
                flush_pend()
                while fi < len(fillers):
                    fillers[fi][1]()
                    fi += 1

            # ---- emission schedule ----
            # stage 0 (before attention): only the minimal projections the
            # first exp needs -- K/V and Q over xT cols 0:256.  Everything
            # else streams in as deadline-fillers inside the attention loop.
            emit_kv(0, 256)
            emit_q(0, 256)

            # attention c0 bins: [A1],[t1,t7],[t2,A2],[t3,t6],[B,t5],[t4],
            # then 5 other bins (tiles 16..23).  Fillers follow the DMA
            # stream: x1 (Q1/KV1), x0b (Qb/KVb), Vtrans, other-c0 chunks 4,5,
            # own-c1 chunks 2,3 + Q c1 (needed only by chunk 1).
            emit_chunk(
                0,
                [
                    (1, lambda: emit_q(256, 512)),
                    (1, lambda: emit_q(512, 768)),
                    (1, lambda: emit_vtrans(0, 2)),
                    (2, lambda: emit_kv(768, 1024)),
                    (2, lambda: emit_q(768, 1024)),
                    (3, lambda: emit_kv(256, 512)),
                    (5, lambda: emit_kv(512, 768)),
                    (5, lambda: emit_vtrans(2, 2)),
                    (6, lambda: emit_vtrans(4)),
                    (6, lambda: emit_kv_chunk(4)),
                    (7, lambda: emit_vtrans(16)),
                    (8, lambda: emit_kv_chunk(5)),
                    (9, lambda: emit_vtrans(20)),
                    (11, lambda: emit_q_chunk(2)),
                    (12, lambda: emit_q_chunk(3)),
                ],
            )

            # attention c1: bins 0-12 own tiles 0..15 (Vn 8..11 used from bin
            # 8, 12..15 from bin 9), bins 13-25 other tiles 16..31 (tiles
            # 24..27 used from bin 21, 28..31 from bin 22 on).
            emit_chunk(
                1,
                [
                    (2, lambda: emit_kv_chunk(2)),
                    (4, lambda: emit_vtrans(8)),
                    (5, lambda: emit_kv_chunk(3)),
                    (7, lambda: emit_vtrans(12)),
                    (10, lambda: emit_kv_chunk(6)),
                    (12, lambda: emit_vtrans(24)),
                    (14, lambda: emit_kv_chunk(7)),
                    (16, lambda: emit_vtrans(28)),
                ],
            )

    nc.compile()
    return nc


def _host_inputs(x, Wq, Wk, Wv):
    """Build the 8 per-core input maps (host-side layout prep only)."""
    tri = np.triu(np.ones((128, 128), np.float32)).astype(BF16)
    ones = np.ones((128, 128), BF16)
    zeros = np.zeros((128, 128), BF16)
    # weights packed [128, 4, 192]: row a*128+p -> [p, a, (wq|wk|wv)]
    w = np.empty((128, 4, 192), np.float32)
    wq = Wq.reshape(4, 128, H)
    wk = Wk.reshape(4, 128, H)
    wv = Wv.reshape(4, 128, H)
    w[:, :, 0:64] = wq.transpose(1, 0, 2)
    w[:, :, 64:128] = wk.transpose(1, 0, 2)
    w[:, :, 128:192] = wv.transpose(1, 0, 2)
    w = w.reshape(128, 4 * 192).astype(BF16)

    in_maps = []
    for b in range(B):
        xb = x[b].reshape(NK, 128, C)
        for par in (0, 1):
            # own parity blocks first, then the other parity's
            perm = np.concatenate(
                [np.arange(par, NK, 2), np.arange(1 - par, NK, 2)]
            )
            xp = xb[perm].reshape(T, C)      # [T, C] permuted rows
            # device layout [p, t, a]: element = xT_perm[a*128+p, t]
            xT = np.ascontiguousarray(
                xp.T.reshape(4, 128, T).transpose(1, 2, 0).reshape(128, T * 4)
            ).astype(BF16)
            msk = np.concatenate(
                [tri, zeros if par == 0 else ones], axis=1
            ).astype(BF16)
            in_maps.append({"xT": xT, "w": w, "msk": msk})
    return in_maps


def kernel(x, Wq, Wk, Wv, _want_trace=False):
    from concourse.bass_utils import run_bass_kernel_spmd

    x = np.asarray(x, dtype=np.float32)
    Wq = np.asarray(Wq, dtype=np.float32)
    Wk = np.asarray(Wk, dtype=np.float32)
    Wv = np.asarray(Wv, dtype=np.float32)

    if "nc" not in _PROGRAM_CACHE:
        _PROGRAM_CACHE["nc"] = build_program()
    nc = _PROGRAM_CACHE["nc"]

    in_maps = _host_inputs(x, Wq, Wk, Wv)
    res = run_bass_kernel_spmd(
        nc, in_maps, core_ids=list(range(NCORES)), trace=_want_trace
    )

    out = np.zeros((B, T, H), np.float32)
    for b in range(B):
        for par in (0, 1):
            r = np.asarray(res.results[2 * b + par]["out"], np.float32)
            r = r.reshape(128, NJ, H).transpose(1, 0, 2)
            out[b].reshape(NK, 128, H)[par::2] = r
    if _want_trace:
        return out, res
    return out
